# revision 6
# baseline (speedup 1.0000x reference)
"""NetVLAD Trainium2 Bass kernel, SPMD over 8 NeuronCores.

Contract: kernel(x, Wc, C) takes the FULL inputs
  x  [16, 56, 56, 512] f32, Wc [512, 32] f32, C [512, 32] f32
and returns the FULL output [16, 16384] f32 (matches reference()).

Sharding: data-parallel over batch - 2 samples per core; Wc/C replicated.

Design (3136 pixels/sample, D=512, K=32; 49 tiles of 128 pixels,
processed in 12 groups of 4 tiles + 1 runt tile):
  - x is cast to fp8_e3m4 on the HOST (4 mantissa bits cover N(0,1)
    activations; measured end-to-end rel err ~8e-3 vs f32 reference,
    budget 2e-2), quartering the HBM stream vs f32; Wc / a / C stay
    bf16 (their fp8 quantization measurably breaks the budget)
  - mixed-dtype matmuls: fp8 x against bf16 weights; PE upconverts
  - per-sample VLAD accumulators live in ONE PSUM bank as [d, 4, k]
    regions (plus an asum bank), opened ONCE by a start=True matmul of
    DMA'd zeros; all real matmuls accumulate with start=False
  - mm1 needs x transposed (contract over d); a tunable subset of
    groups (DUAL) streams a host-pretransposed fp8 xT copy instead of
    PE-transposing, balancing the PE and DMA serial resources
  - per group g (4 tiles / 512 pixels):
      T: 16 PE is_transpose matmuls -> xT fp8 PSUM (skipped for DUAL)
      Dr: ONE DVE copy drains xT PSUM->SBUF through a uint16 bitcast
          view (fp8 has no 2x DVE mode; uint16 does)
      M1: 16 accumulating matmuls s = xT.T @ Wc -> [128, 4, 32] f32
      E:  ONE batched ACT Exp over all 4 tiles' logits
      Z:  DVE reduce_sum -> Z [128, 4]; DVE reciprocal -> 1/Z
      N:  ONE Pool tensor_tensor mult (exp * bcast(1/Z)) -> a bf16
      M2: 16 matmuls acc[d,k] += x_chunk.T @ a (+ 4 asum matmuls),
          sample-split partition ranges at the boundary tile 24
  - per-sample epilogue (same math as before): diag(asum) trick folds
    C*asum into PSUM via 4 PE matmuls carrying the stop; Square + DVE
    reduce for ssq; 1/sqrt(D*ssq) via exp(-0.5*ln(.)); DVE scales out
    of PSUM; one 512B-descriptor output DMA per sample
NOTE: dram params MUST receive numpy arrays of the declared dtype
(ml_dtypes); f32 arrays are silently reinterpreted and produce NaN.
"""
import sys

if '/opt/trn_rl_repo' not in sys.path:
    sys.path.insert(0, '/opt/trn_rl_repo')

from contextlib import ExitStack

import numpy as np

N_PIX = 3136
N_SAMP = 2
N_ROWS = N_PIX * N_SAMP
P = 128
NT = N_ROWS // P      # 49
D = 512
K = 32
DC = D // P           # 4
G = 4                 # tiles per group
NG = NT // G          # 12 full groups
RUNT = NT - NG * G    # 1
BOUND_T = N_PIX // P  # 24 (tile 24 is split 64/64 between samples)
BOUND_R = N_PIX - BOUND_T * P  # 64
N_CORES = 8

# groups whose xT comes pre-transposed from the host (DMA) instead of
# PE transpose + drain; tunes the PE-vs-DMA balance. fp8 is_transpose
# is rejected by the walrus verifier, so non-dual groups use normal-mode
# f32-output transposes (fp8 x against a bf16 identity) drained by
# DVE/ACT in halves.
DUAL = tuple(range(13))

_cache = {}


def _group_tiles(g):
    return (G, g * G) if g < NG else (RUNT, NG * G)


def _build(dual=DUAL):
    import concourse.bacc as bacc
    import concourse.mybir as mybir
    import concourse.tile as tile
    from concourse.bass import ts

    F32 = mybir.dt.float32
    BF16 = mybir.dt.bfloat16
    FP8 = mybir.dt.float8e3
    U16 = mybir.dt.uint16

    dual = tuple(sorted(dual))
    n_dual = len(dual)
    dual_idx = {g: i for i, g in enumerate(dual)}

    nc = bacc.Bacc("TRN2", target_bir_lowering=False, debug=False)

    x = nc.declare_dram_parameter("x", [N_ROWS, D], FP8, isOutput=False)
    if n_dual:
        xt = nc.declare_dram_parameter("xt", [n_dual * D, G * P], FP8,
                                       isOutput=False)
    wc = nc.declare_dram_parameter("wc", [D, K], BF16, isOutput=False)
    ct = nc.declare_dram_parameter("ct", [K, D], BF16, isOutput=False)
    ident = nc.declare_dram_parameter("ident", [P, P], FP8, isOutput=False)
    id32 = nc.declare_dram_parameter("id32", [K, K], BF16, isOutput=False)
    zeros = nc.declare_dram_parameter("zeros", [P, 2 * DC * K], BF16,
                                      isOutput=False)
    ones2 = nc.declare_dram_parameter("ones2", [P, 2], BF16, isOutput=False)
    out = nc.declare_dram_parameter("out", [N_SAMP, P, DC, K], F32,
                                    isOutput=True)
    x, wc, ct, ident, out, id32, ones2, zeros = (
        x.ap(), wc.ap(), ct.ap(), ident.ap(), out.ap(), id32.ap(),
        ones2.ap(), zeros.ap())
    if n_dual:
        xt = xt.ap()

    with tile.TileContext(nc) as tc, ExitStack() as ctx:
        consts = ctx.enter_context(tc.tile_pool(name="consts", bufs=1))
        xbig = ctx.enter_context(tc.tile_pool(name="xbig", bufs=1))
        xtp = ctx.enter_context(tc.tile_pool(name="xtp", bufs=3))
        xtd = ctx.enter_context(tc.tile_pool(name="xtd", bufs=1))
        small = ctx.enter_context(tc.tile_pool(name="small", bufs=3))
        epil = ctx.enter_context(tc.tile_pool(name="epil", bufs=2))
        ps_xt = ctx.enter_context(tc.tile_pool(name="ps_xt", bufs=2,
                                               space="PSUM"))
        ps_s = ctx.enter_context(tc.tile_pool(name="ps_s", bufs=3,
                                              space="PSUM"))
        ps_acc = ctx.enter_context(tc.tile_pool(name="ps_acc", bufs=1,
                                                space="PSUM"))

        # constants on the gpsimd SWDGE queue (parallel with the x
        # stream on SP); id_sb first since the first transposes need it
        id_sb = consts.tile([P, P], FP8)
        nc.gpsimd.dma_start(out=id_sb, in_=ident)
        wc_sb = consts.tile([P, DC, K], BF16)
        nc.gpsimd.dma_start(out=wc_sb, in_=wc.rearrange("(c p) k -> p c k",
                                                        p=P))
        ones_sb = consts.tile([P, 2], BF16)
        nc.gpsimd.dma_start(out=ones_sb, in_=ones2)
        ct_sb = consts.tile([K, D], BF16)
        nc.gpsimd.dma_start(out=ct_sb, in_=ct)
        id32_sb = consts.tile([K, K], BF16)
        nc.gpsimd.dma_start(out=id32_sb, in_=id32)
        zeros_sb = consts.tile([P, 2 * DC * K], BF16)
        nc.gpsimd.dma_start(out=zeros_sb, in_=zeros)

        # pin the one act table covering Exp+Ln up front so the fixpoint
        # pass doesn't thrash between exp-only and ln-only sets at the
        # epilogues; purely a perf hint
        try:
            from concourse.hw_specs import get_activation_tables
            sets = get_activation_tables(nc.m.arch)
            _EXP = mybir.ActivationFunctionType.Exp
            _LN = mybir.ActivationFunctionType.Ln
            set_id = next(i for i, name in enumerate(sets)
                          if _EXP in sets[name] and _LN in sets[name])
            nc.scalar.add_instruction(
                mybir.InstLoadActFuncSet(
                    name=nc.get_next_instruction_name(), ins=[], outs=[],
                    act_func_set_id=set_id))
        except Exception:
            pass

        # x resident in SBUF, one tile per group; group 0 split in
        # halves along d so the first transposes start as soon as the
        # first columns land
        xg = []
        for g in range(NG + 1):
            n, t0 = _group_tiles(g)
            t_ = xbig.tile([P, n, D], FP8, name=f"xg{g}")
            xg.append(t_)

        def dma_x_group(g):
            n, t0 = _group_tiles(g)
            r0 = t0 * P
            if g == 0:
                for h in range(2):
                    hd = slice(h * D // 2, (h + 1) * D // 2)
                    nc.sync.dma_start(out=xg[0][:, 0, hd],
                                      in_=x[r0:r0 + P, hd])
                nc.sync.dma_start(
                    out=xg[0][:, 1:n, :],
                    in_=x[r0 + P:r0 + n * P, :].rearrange(
                        "(j p) d -> p j d", p=P))
            else:
                nc.sync.dma_start(
                    out=xg[g],
                    in_=x[r0:r0 + n * P, :].rearrange("(j p) d -> p j d",
                                                      p=P))

        xt_tiles = {}

        def dma_xt_group(g):
            # host-pretransposed xT block for a dual group, chunk-major
            # [128, c, pix]; 512B descriptors (runt block is narrower)
            n, t0 = _group_tiles(g)
            i = dual_idx[g]
            t_ = xtd.tile([P, DC, n * P], FP8, name=f"xt{g}")
            nc.scalar.dma_start(
                out=t_,
                in_=xt[i * D:(i + 1) * D, 0:n * P].rearrange(
                    "(c p) q -> p c q", p=P))
            xt_tiles[g] = t_

        # one PSUM bank holds both samples' [d, 4, k] accumulators in
        # disjoint column ranges; a second holds the two asum pairs,
        # opened ONCE by a start=True matmul of DMA'd zeros
        ps_all = ps_acc.tile([P, 2 * DC * K], F32, name="ps_all", tag="acc")
        asums = ps_acc.tile([K, 4], F32, name="asums", tag="asum")
        nc.tensor.matmul(ps_all, id_sb[:, 0:P].bitcast(FP8), zeros_sb,
                         start=True, stop=False, skip_group_check=True)
        nc.tensor.matmul(asums, id_sb[:, 0:K].bitcast(FP8), zeros_sb[:, 0:4],
                         start=True, stop=False, skip_group_check=True)

        def acc_jk(s, j):
            o = (s * DC + j) * K
            return ps_all[:, o:o + K]

        def acc_hj(s, h):
            o = (s * DC + 2 * h) * K
            return ps_all[:, o:o + 2 * K]

        asum_ps = [asums[:, 2 * s:2 * s + 2] for s in range(N_SAMP)]

        diags = {}

        def epilogue_pre(s):
            asum_sb = epil.tile([K, 1], F32, name=f"asum{s}", tag="asum")
            nc.vector.tensor_copy(asum_sb, asum_ps[s][:, 0:1])
            # diag(asum) = id32 * asum (per-partition scalar), bf16 for PE
            diag = epil.tile([K, K], BF16, name=f"diag{s}", tag="diag")
            nc.vector.tensor_scalar_mul(diag, id32_sb, asum_sb)
            diags[s] = diag

        def epilogue_post(s):
            # fold the C*asum term into the PSUM accumulation:
            # acc[s] chunk j += ct_chunk.T @ diag(asum); these carry the
            # stop of the accumulation groups
            diag = diags.pop(s)
            for j in range(DC):
                nc.tensor.matmul(acc_jk(s, j), ct_sb[:, ts(j, P)], diag,
                                 start=False, stop=True,
                                 skip_group_check=True)
            v_sb = epil.tile([P, DC, K], F32, name=f"v{s}", tag="v")
            vsq = epil.tile([P, DC, K], F32, name=f"vsq{s}", tag="vsq")
            ssq = epil.tile([P, DC], F32, name=f"ssq{s}", tag="ssq")
            lssq = epil.tile([P, DC], F32, name=f"ls{s}", tag="ls")
            rmult = epil.tile([P, DC], F32, name=f"rm{s}", tag="rm")
            # rmult = exp(-0.5*ln(D*ssq)) = 1/sqrt(D*ssq); ln+exp share
            # one act func set (no Sqrt set switch)
            for h in range(2):
                hj = slice(2 * h, 2 * h + 2)
                nc.scalar.activation(vsq[:, hj, :], acc_hj(s, h),
                                     mybir.ActivationFunctionType.Square)
                nc.vector.reduce_sum(ssq[:, hj], vsq[:, hj, :],
                                     axis=mybir.AxisListType.X)
            nc.scalar.activation(lssq, ssq,
                                 mybir.ActivationFunctionType.Ln,
                                 scale=float(D))
            nc.scalar.activation(rmult, lssq,
                                 mybir.ActivationFunctionType.Exp,
                                 scale=-0.5)
            for j in range(DC):
                nc.vector.tensor_scalar_mul(v_sb[:, j, :], acc_jk(s, j),
                                            rmult[:, j:j + 1])
            qeng = nc.sync if s == 1 else nc.gpsimd
            qeng.dma_start(out=out[s], in_=v_sb)

        xt_sbs = {}
        a_sbs = {}

        def stage_T(g):
            # PE transposes x tiles into a chunk-major fp8 PSUM bank;
            # ONE DVE instr drains it through a uint16 view (2x mode)
            n, t0 = _group_tiles(g)
            xT_ps = ps_xt.tile([P, DC, n * P], FP8, name="xT_ps")
            xT_sb = xtp.tile([P, DC, n * P], FP8, name="xT_sb")
            for i in range(n):
                for c in range(DC):
                    nc.tensor.transpose(xT_ps[:, c, ts(i, P)],
                                        xg[g][:, i, ts(c, P)], id_sb)
            nc.vector.tensor_copy(xT_sb.bitcast(U16), xT_ps.bitcast(U16))
            xt_sbs[g] = xT_sb

        def stage_M1(g):
            n, t0 = _group_tiles(g)
            xT_sb = xt_tiles.pop(g) if g in dual_idx else xt_sbs.pop(g)
            s_ps = ps_s.tile([P, n, K], F32, name="s_ps", tag="sps")
            for i in range(n):
                for c in range(DC):
                    nc.tensor.matmul(s_ps[:, i, :], xT_sb[:, c, ts(i, P)],
                                     wc_sb[:, c, :],
                                     start=(c == 0), stop=(c == DC - 1))
            exp_sb = small.tile([P, n, K], F32, name="exp_sb")
            zs = small.tile([P, n], F32, name="zs")
            rz = small.tile([P, n, 1], F32, name="rz")
            a_sb = small.tile([P, n, K], BF16, name="a_sb")
            nc.scalar.activation(exp_sb, s_ps,
                                 mybir.ActivationFunctionType.Exp)
            nc.vector.reduce_sum(zs, exp_sb, axis=mybir.AxisListType.X)
            nc.vector.reciprocal(rz[:, :, 0], zs)
            # softmax divide: one batched mult on the idle Pool engine,
            # 1/Z broadcast along k via a stride-0 AP
            nc.gpsimd.tensor_tensor(a_sb, exp_sb,
                                    rz.broadcast_to([P, n, K]),
                                    op=mybir.AluOpType.mult)
            a_sbs[g] = a_sb

        def stage_M2(g):
            n, t0 = _group_tiles(g)
            a_sb = a_sbs.pop(g)
            for i in range(n):
                t = t0 + i
                if t < BOUND_T:
                    parts = [(0, 0, P)]
                elif t == BOUND_T:
                    parts = [(0, 0, BOUND_R), (1, BOUND_R, P)]
                else:
                    parts = [(1, 0, P)]
                for s, r0, r1 in parts:
                    last_tile = (t == BOUND_T and s == 0) or \
                                (t == NT - 1 and s == 1)
                    for c in range(DC):
                        # acc[d, k] += x_chunk.T @ a -- output born in
                        # [d, k] layout, no epilogue back-transpose;
                        # the C*asum matmuls carry the stop
                        nc.tensor.matmul(acc_jk(s, c),
                                         xg[g][r0:r1, i, ts(c, P)],
                                         a_sb[r0:r1, i, :],
                                         start=False, stop=False,
                                         skip_group_check=True)
                    nc.tensor.matmul(asum_ps[s][:, :], a_sb[r0:r1, i, :],
                                     ones_sb[r0:r1, :],
                                     start=False, stop=last_tile,
                                     skip_group_check=True)
                    if last_tile:
                        epilogue_pre(s)

        # prefetch: groups 0-2 x up front, dual xT as needed
        for g in range(min(3, NG + 1)):
            dma_x_group(g)
        for g in dual:
            if g <= 3:
                dma_xt_group(g)

        for t in range(NG + 3):
            if t + 3 <= NG:
                dma_x_group(t + 3)
            if t + 4 in dual_idx:
                dma_xt_group(t + 4)
            if 0 <= t - 2 <= NG:
                stage_M2(t - 2)
            if t <= NG and t not in dual_idx:
                stage_T(t)
            if 0 <= t - 1 <= NG:
                stage_M1(t - 1)
            if t - 2 == BOUND_T // G + 2:
                epilogue_post(0)
            if t - 2 == NG:
                epilogue_post(1)

    nc.finalize()
    return nc


def _get_nc():
    if "nc" not in _cache:
        _cache["nc"] = _build()
    return _cache["nc"]


def _make_maps(x, Wc, C, dual=DUAL):
    import ml_dtypes
    bf16 = ml_dtypes.bfloat16
    fp8 = ml_dtypes.float8_e3m4
    x = np.asarray(x, dtype=np.float32)
    Wc = np.ascontiguousarray(np.asarray(Wc, dtype=np.float32).astype(bf16))
    ct = np.ascontiguousarray(np.asarray(C, dtype=np.float32).T.astype(bf16))
    ident = np.eye(P, dtype=np.float32).astype(fp8)
    id32 = np.eye(K, dtype=np.float32).astype(bf16)
    ones2 = np.ones((P, 2), dtype=np.float32).astype(bf16)
    zeros = np.zeros((P, 2 * DC * K), dtype=np.float32).astype(bf16)

    B = x.shape[0]
    per = B // N_CORES
    maps = []
    for i in range(N_CORES):
        xs = np.ascontiguousarray(
            x[i * per:(i + 1) * per].reshape(N_ROWS, D).astype(fp8))
        m = {"x": xs, "wc": Wc, "ct": ct, "ident": ident,
             "id32": id32, "ones2": ones2, "zeros": zeros}
        if dual:
            blocks = []
            for g in sorted(dual):
                blk = xs[g * G * P:min((g + 1) * G * P, N_ROWS)].T
                if blk.shape[1] < G * P:
                    pad = np.zeros((D, G * P - blk.shape[1]), dtype=blk.dtype)
                    blk = np.concatenate([blk, pad], axis=1)
                blocks.append(np.ascontiguousarray(blk))
            m["xt"] = np.concatenate(blocks, axis=0)
        maps.append(m)
    return maps


def kernel(x, Wc, C):
    from concourse.bass_utils import run_bass_kernel_spmd

    nc = _get_nc()
    maps = _make_maps(x, Wc, C)
    res = run_bass_kernel_spmd(nc, maps, list(range(N_CORES)))
    outs = [r["out"].transpose(0, 2, 1, 3).reshape(N_SAMP, D * K)
            for r in res.results]
    return np.concatenate(outs, axis=0)


# revision 27
# speedup vs baseline: 1.7895x; 1.7895x over previous
"""NetVLAD Trainium2 Bass kernel, SPMD over 8 NeuronCores.

Contract: kernel(x, Wc, C) takes the FULL inputs
  x  [16, 56, 56, 512] f32, Wc [512, 32] f32, C [512, 32] f32
and returns the FULL output [16, 16384] f32 (matches reference()).

Sharding: data-parallel over batch - 2 samples per core; Wc/C replicated.

Design (3136 pixels/sample, D=512, K=32; 49 tiles of 128 pixels in 12
groups of 4 + 1 runt):
  - x is cast to fp8_e3m4 on the HOST (4 mantissa bits cover N(0,1)
    activations; end-to-end rel err ~8e-3 vs the f32 reference, budget
    2e-2); Wc / a / C stay bf16 (fp8 weights measurably break the
    budget). Matmuls mix fp8 stationary x with bf16 moving operands.
  - mm1 (logits, contracts d) needs x with d on partitions; mm2 (VLAD
    accumulation, contracts pixels) needs pixels on partitions. fp8 PE
    transposes are rejected by the walrus verifier and f32-output
    transposes cost more drain bandwidth than they save, so BOTH
    layouts stream from the host: x natural and xT pre-transposed,
    fp8 each (2 x 3.2 MB/core).
  - DMA time is charged per issuing queue (SP / Activation / Pool are
    the three DMA-capable queues), so the 26 stream DMAs are spread by
    a greedy balancer seeded with each queue's compute load; all
    constants are packed into ONE bf16 dram param (single 500ns DMA).
  - per-sample VLAD accumulators live in ONE PSUM bank as [d, 4, k]
    regions plus an asum bank, zeroed ONCE by DVE memsets; every
    matmul accumulates with start=False (skip_group_check).
  - softmax is batched per PAIR of groups (8 tiles / 1024 pixels):
    ONE ACT Exp [128, 8, 32] straight out of the mm1 PSUM pair-tile,
    ONE DVE reduce -> Z, ONE DVE reciprocal, ONE Pool tensor_tensor
    multiply with 1/Z broadcast via a stride-0 AP -> a bf16.
  - mm2: 16 matmuls/group acc[d,k] += x_chunk.T @ a plus asum matmuls,
    sample-split partition ranges at boundary tile 24.
  - epilogue per sample: diag(asum) trick folds C*asum into PSUM via 4
    PE matmuls carrying the stop; ONE Square + ONE DVE reduce for ssq;
    1/sqrt(D*ssq) = exp(-0.5*ln(D*ssq)) on ACT (shared Exp+Ln table,
    pinned once); ONE broadcast tensor_tensor scales straight out of
    PSUM; one 512B-per-partition output DMA per sample.
NOTE: dram params MUST receive numpy arrays of the declared dtype
(ml_dtypes); f32 arrays are silently reinterpreted and produce NaN.
"""
import sys

if '/opt/trn_rl_repo' not in sys.path:
    sys.path.insert(0, '/opt/trn_rl_repo')

from contextlib import ExitStack

import numpy as np

N_PIX = 3136
N_SAMP = 2
N_ROWS = N_PIX * N_SAMP
P = 128
NT = N_ROWS // P      # 49
D = 512
K = 32
DC = D // P           # 4
G = 4                 # tiles per group
NG = NT // G          # 12 full groups
RUNT = NT - NG * G    # 1
BOUND_T = N_PIX // P  # 24 (tile 24 is split 64/64 between samples)
BOUND_R = N_PIX - BOUND_T * P  # 64
N_CORES = 8

# packed-constants column layout (bf16 elements)
CPK_WC = 0            # [128, 128]: wc[p, c*32+k] = Wc[c*128+p, k]
CPK_CT = 128          # [32, 512]:  ct[k, d] = C[d, k]
CPK_ID32 = 640        # [32, 32]:   eye
CPK_ONES = 672        # [128, 2]:   ones
CPK_W = 674

_cache = {}


def _group_tiles(g):
    return (G, g * G) if g < NG else (RUNT, NG * G)


def _build():
    import concourse.bacc as bacc
    import concourse.mybir as mybir
    import concourse.tile as tile
    from concourse.bass import ts

    F32 = mybir.dt.float32
    BF16 = mybir.dt.bfloat16
    FP8 = mybir.dt.float8e3

    nc = bacc.Bacc("TRN2", target_bir_lowering=False, debug=False)

    x = nc.declare_dram_parameter("x", [N_ROWS, D], FP8, isOutput=False)
    xt = nc.declare_dram_parameter("xt", [(NG + 1) * D, G * P], FP8,
                                   isOutput=False)
    cpk = nc.declare_dram_parameter("cpk", [P, CPK_W], BF16, isOutput=False)
    out = nc.declare_dram_parameter("out", [N_SAMP, P, DC, K], F32,
                                    isOutput=True)
    x, xt, cpk, out = x.ap(), xt.ap(), cpk.ap(), out.ap()

    # greedy DMA queue balancer, seeded with each queue's compute load
    # (scalar carries the exps + act-table loads + epilogue activations;
    # gpsimd only the consts; sync only an output half)
    qload = dict(_cache.get("seeds", {"sync": 500.0, "scalar": 3000.0,
                                      "gpsimd": 1000.0}))

    def dma(out_, in_, cost, queue=None):
        if queue is None:
            queue = min(qload, key=qload.get)
        qload[queue] += max(cost, 500.0)
        getattr(nc, queue).dma_start(out=out_, in_=in_)

    with tile.TileContext(nc) as tc, ExitStack() as ctx:
        consts = ctx.enter_context(tc.tile_pool(name="consts", bufs=1))
        xbig = ctx.enter_context(tc.tile_pool(name="xbig", bufs=1))
        xtd = ctx.enter_context(tc.tile_pool(name="xtd", bufs=1))
        small = ctx.enter_context(tc.tile_pool(name="small", bufs=4))
        epil = ctx.enter_context(tc.tile_pool(name="epil", bufs=2))
        ps_s = ctx.enter_context(tc.tile_pool(name="ps_s", bufs=3,
                                              space="PSUM"))
        ps_acc = ctx.enter_context(tc.tile_pool(name="ps_acc", bufs=1,
                                                space="PSUM"))

        # one packed bf16 constants DMA; everything else is a view
        cpk_sb = consts.tile([P, CPK_W], BF16)
        dma(cpk_sb, cpk, CPK_W * 2 * 0.3855, queue="gpsimd")
        wc_sb = cpk_sb[:, CPK_WC:CPK_WC + DC * K].rearrange(
            "p (c k) -> p c k", c=DC)
        ct_sb = cpk_sb[0:K, CPK_CT:CPK_CT + D]
        id32_sb = cpk_sb[0:K, CPK_ID32:CPK_ID32 + K]
        ones_sb = cpk_sb[:, CPK_ONES:CPK_ONES + 2]

        xg = []
        xt_tiles = []
        for g in range(NG + 1):
            n, t0 = _group_tiles(g)
            xg.append(xbig.tile([P, n, D], FP8, name=f"xg{g}"))
            xt_tiles.append(xtd.tile([P, DC, n * P], FP8, name=f"xt{g}"))

        def dma_group(g, queues=(None, None)):
            n, t0 = _group_tiles(g)
            r0 = t0 * P
            dma(xt_tiles[g],
                xt[g * D:(g + 1) * D, 0:n * P].rearrange(
                    "(c p) q -> p c q", p=P),
                DC * n * P * 0.3855 * (1 if n == G else 2),
                queue=queues[1])
            dma(xg[g],
                x[r0:r0 + n * P, :].rearrange("(j p) d -> p j d", p=P),
                n * D * 0.3855, queue=queues[0])

        # per-sample [d, 4, k] accumulators as SEPARATE PSUM tiles (so
        # sample 0's epilogue readers don't false-serialize against
        # sample 1's accumulation writers), zeroed once by DVE memsets;
        # all matmuls accumulate with start=False (per-element HW
        # semantics match plain writes)
        ps_acc_t = [ps_acc.tile([P, DC * K], F32, name=f"acc{s}",
                                tag=f"acc{s}") for s in range(N_SAMP)]
        asums_t = [ps_acc.tile([K, 2], F32, name=f"asum{s}",
                               tag=f"asumps{s}") for s in range(N_SAMP)]
        for s in range(N_SAMP):
            nc.vector.memset(ps_acc_t[s], 0.0)
            nc.vector.memset(asums_t[s], 0.0)

        def acc_jk(s, j):
            return ps_acc_t[s][:, j * K:(j + 1) * K]

        def acc_s(s):
            return ps_acc_t[s].rearrange("p (c k) -> p c k", c=DC)

        asum_ps = asums_t

        diags = {}

        def epilogue_pre(s):
            asum_sb = epil.tile([K, 1], F32, name=f"asum{s}", tag="asum")
            nc.vector.tensor_copy(asum_sb, asum_ps[s][:, 0:1])
            # diag(asum) = id32 * asum (per-partition scalar), bf16 for PE
            diag = epil.tile([K, K], BF16, name=f"diag{s}", tag="diag")
            nc.vector.tensor_scalar_mul(diag, id32_sb, asum_sb)
            diags[s] = diag

        def epilogue_post(s):
            # fold the C*asum term into the PSUM accumulation:
            # acc[s] chunk j += ct_chunk.T @ diag(asum); these carry the
            # stop of the accumulation groups
            diag = diags.pop(s)
            for j in range(DC):
                nc.tensor.matmul(acc_jk(s, j), ct_sb[:, ts(j, P)], diag,
                                 start=False, stop=True,
                                 skip_group_check=True)
            # rmult = exp(-0.5*ln(D*ssq)) = 1/sqrt(D*ssq); ln+exp share
            # one act func set (no Sqrt set switch). Sample 0 runs one
            # monolithic chain (fewer ACT ops queued ahead of the
            # remaining loop exps on the in-order ACT queue); sample 1
            # -- the tail -- runs two pipelined half-chains so the
            # first output half DMAs while the second still scales.
            v_sb = epil.tile([P, DC, K], F32, name=f"v{s}", tag="v")
            vsq = epil.tile([P, DC, K], F32, name=f"vsq{s}", tag="vsq")
            ssq = epil.tile([P, DC], F32, name=f"ssq{s}", tag="ssq")
            lssq = epil.tile([P, DC], F32, name=f"ls{s}", tag="ls")
            rmult = epil.tile([P, DC, 1], F32, name=f"rm{s}", tag="rm")
            halves = [slice(0, DC)]
            if s == 1 and _cache.get("split_s1", False):
                halves = [slice(0, 2), slice(2, DC)]
            for h, hj in enumerate(halves):
                w = hj.stop - hj.start
                nc.scalar.activation(vsq[:, hj, :], acc_s(s)[:, hj, :],
                                     mybir.ActivationFunctionType.Square)
                nc.vector.reduce_sum(ssq[:, hj], vsq[:, hj, :],
                                     axis=mybir.AxisListType.X)
                nc.scalar.activation(lssq[:, hj], ssq[:, hj],
                                     mybir.ActivationFunctionType.Ln,
                                     scale=float(D))
                nc.scalar.activation(rmult[:, hj, 0], lssq[:, hj],
                                     mybir.ActivationFunctionType.Exp,
                                     scale=-0.5)
                nc.vector.tensor_tensor(v_sb[:, hj, :], acc_s(s)[:, hj, :],
                                        rmult[:, hj, :].broadcast_to(
                                            [P, w, K]),
                                        op=mybir.AluOpType.mult)
                dma(out[s][:, hj, :], v_sb[:, hj, :], 500,
                    queue=("gpsimd", "sync")[h] if s == 1 else "gpsimd")

        SB = 4  # groups per softmax batch
        s_pairs = {}
        a_pairs = {}

        def stage_M1(g):
            n, t0 = _group_tiles(g)
            p, half = g // SB, g % SB
            if p not in s_pairs:
                w = SB * G if g < NG else RUNT
                s_pairs[p] = ps_s.tile([P, w, K], F32, name="spair",
                                       tag="sp")
            s_ps = s_pairs[p]
            xT_sb = xt_tiles[g]
            for i in range(n):
                for c in range(DC):
                    nc.tensor.matmul(s_ps[:, half * G + i, :],
                                     xT_sb[:, c, ts(i, P)],
                                     wc_sb[:, c, :],
                                     start=(c == 0), stop=(c == DC - 1))

        def stage_SM(p):
            # batched softmax over four groups (16 tiles): one Exp, one
            # reduce, one reciprocal, one broadcast multiply
            s_ps = s_pairs.pop(p)
            w = s_ps.shape[1]
            exp_sb = small.tile([P, w, K], F32, name="exp_sb")
            zs = small.tile([P, w], F32, name="zs")
            rz = small.tile([P, w, 1], F32, name="rz")
            a_sb = small.tile([P, w, K], BF16, name="a_sb")
            nc.scalar.activation(exp_sb, s_ps,
                                 mybir.ActivationFunctionType.Exp)
            # Z, 1/Z and the divide all stay on DVE: back-to-back on one
            # queue beats hopping to Pool (two fewer semaphore waits on
            # the a-chain), and it frees Pool for pure DMA work
            nc.vector.reduce_sum(zs, exp_sb, axis=mybir.AxisListType.X)
            nc.vector.reciprocal(rz[:, :, 0], zs)
            norm_eng = nc.vector if _cache.get("norm_dve", False) \
                else nc.gpsimd
            norm_eng.tensor_tensor(a_sb, exp_sb,
                                   rz.broadcast_to([P, w, K]),
                                   op=mybir.AluOpType.mult)
            a_pairs[p] = a_sb

        def stage_M2(g):
            n, t0 = _group_tiles(g)
            p, half = g // SB, g % SB
            a_sb = a_pairs[p]
            for i in range(n):
                t = t0 + i
                ai = a_sb[:, half * G + i, :]
                if t < BOUND_T:
                    parts = [(0, 0, P)]
                elif t == BOUND_T:
                    parts = [(0, 0, BOUND_R), (1, BOUND_R, P)]
                else:
                    parts = [(1, 0, P)]
                for s, r0, r1 in parts:
                    last_tile = (t == BOUND_T and s == 0) or \
                                (t == NT - 1 and s == 1)
                    def asum_mm():
                        nc.tensor.matmul(asum_ps[s][:, :], ai[r0:r1, :],
                                         ones_sb[r0:r1, :],
                                         start=False, stop=last_tile,
                                         skip_group_check=True)
                        if last_tile:
                            epilogue_pre(s)

                    def acc_mms():
                        for c in range(DC):
                            nc.tensor.matmul(acc_jk(s, c),
                                             xg[g][r0:r1, i, ts(c, P)],
                                             ai[r0:r1, :],
                                             start=False, stop=False,
                                             skip_group_check=True)
                    # asum first: on the last tile the DVE diag chain
                    # (epilogue_pre) overlaps the remaining acc matmuls
                    if _cache.get("asum_first", True):
                        asum_mm(); acc_mms()
                    else:
                        acc_mms(); asum_mm()

        # spread the first groups across all three queues so the pipe
        # fills at full rate; xT leads since mm1 consumes it first, and
        # xt0 goes on SP because the scheduler runs the act-table pin
        # first on ACT (1.3us) regardless of emission order
        dma_group(0, ("scalar", "sync"))
        dma_group(1, ("gpsimd", "sync"))
        dma_group(2, ("scalar", "gpsimd"))

        # pin the one act table covering Exp+Ln+Square up front so the
        # fixpoint table-load pass doesn't thrash between the exp-only
        # and ln-only sets at the epilogues (without this it inserts 5
        # loads, 6.4us of ACT time)
        try:
            from concourse.hw_specs import get_activation_tables
            sets = get_activation_tables(nc.m.arch)
            _EXP = mybir.ActivationFunctionType.Exp
            _LN = mybir.ActivationFunctionType.Ln
            set_id = next(i for i, name in enumerate(sets)
                          if _EXP in sets[name] and _LN in sets[name])
            nc.scalar.add_instruction(
                mybir.InstLoadActFuncSet(
                    name=nc.get_next_instruction_name(), ins=[], outs=[],
                    act_func_set_id=set_id))
        except Exception:
            pass

        # software pipeline: M1 leads, M2 trails by 4 slots so the PE
        # always has M1 matmuls queued while a pair's softmax chain
        # (exp -> Z -> 1/Z -> normalize) hops across ACT/DVE/Pool
        for t in range(NG + 6):
            if 0 <= t - 1 <= NG:
                g = t - 1
                stage_M1(g)
                if g % 4 == 3 or g == NG:
                    stage_SM(g // 4)
            if 0 <= t - 5 <= NG:
                stage_M2(t - 5)
            if t + 3 <= NG:
                dma_group(t + 3)
            if t - 5 == BOUND_T // G + 2:
                epilogue_post(0)
            if t - 5 == NG:
                epilogue_post(1)

    # the single pinned set-6 load covers every activation we emit
    # (Exp, Ln, Square); the auto pass would add a redundant 1.3us load
    nc.insert_act_table_loads = lambda: None
    nc.finalize()
    return nc


def _get_nc():
    if "nc" not in _cache:
        _cache["nc"] = _build()
    return _cache["nc"]


def _make_maps(x, Wc, C):
    import ml_dtypes
    bf16 = ml_dtypes.bfloat16
    fp8 = ml_dtypes.float8_e3m4
    x = np.asarray(x, dtype=np.float32)
    Wc = np.asarray(Wc, dtype=np.float32)
    C = np.asarray(C, dtype=np.float32)

    cpk = np.zeros((P, CPK_W), dtype=np.float32)
    cpk[:, CPK_WC:CPK_WC + DC * K] = Wc.reshape(DC, P, K).transpose(
        1, 0, 2).reshape(P, DC * K)
    cpk[0:K, CPK_CT:CPK_CT + D] = C.T
    cpk[0:K, CPK_ID32:CPK_ID32 + K] = np.eye(K, dtype=np.float32)
    cpk[:, CPK_ONES:CPK_ONES + 2] = 1.0
    cpk = cpk.astype(bf16)

    B = x.shape[0]
    per = B // N_CORES
    maps = []
    for i in range(N_CORES):
        xs = np.ascontiguousarray(
            x[i * per:(i + 1) * per].reshape(N_ROWS, D).astype(fp8))
        blocks = []
        for g in range(NG + 1):
            blk = xs[g * G * P:min((g + 1) * G * P, N_ROWS)].T
            if blk.shape[1] < G * P:
                pad = np.zeros((D, G * P - blk.shape[1]), dtype=blk.dtype)
                blk = np.concatenate([blk, pad], axis=1)
            blocks.append(np.ascontiguousarray(blk))
        maps.append({"x": xs, "xt": np.concatenate(blocks, axis=0),
                     "cpk": cpk})
    return maps


def kernel(x, Wc, C):
    from concourse.bass_utils import run_bass_kernel_spmd

    nc = _get_nc()
    maps = _make_maps(x, Wc, C)
    res = run_bass_kernel_spmd(nc, maps, list(range(N_CORES)))
    outs = [r["out"].transpose(0, 2, 1, 3).reshape(N_SAMP, D * K)
            for r in res.results]
    return np.concatenate(outs, axis=0)


# revision 40
# speedup vs baseline: 1.9188x; 1.0723x over previous
"""NetVLAD Trainium2 Bass kernel, SPMD over 8 NeuronCores.

Contract: kernel(x, Wc, C) takes the FULL inputs
  x  [16, 56, 56, 512] f32, Wc [512, 32] f32, C [512, 32] f32
and returns the FULL output [16, 16384] f32 (matches reference()).

Sharding: data-parallel over batch - 2 samples per core; Wc/C replicated.

Design (3136 pixels/sample, D=512, K=32; 49 tiles of 128 pixels in 12
groups of 4 + 1 runt):
  - x is cast to fp8_e3m4 on the HOST (4 mantissa bits cover N(0,1)
    activations; end-to-end rel err ~8e-3 vs the f32 reference, budget
    2e-2); Wc / a / C stay bf16 (fp8 weights measurably break the
    budget). Matmuls mix fp8 stationary x with bf16 moving operands.
  - mm1 (logits, contracts d) needs x with d on partitions; mm2 (VLAD
    accumulation, contracts pixels) needs pixels on partitions. fp8 PE
    transposes are rejected by the walrus verifier and f32-output
    transposes cost more drain bandwidth than they save, so BOTH
    layouts stream from the host: x natural and xT pre-transposed,
    fp8 each (2 x 3.2 MB/core).
  - DMA time is charged per issuing queue (SP / Activation / Pool are
    the three DMA-capable queues), so the 26 stream DMAs are spread by
    a greedy balancer seeded with each queue's compute load; all
    constants are packed into ONE bf16 dram param (single 500ns DMA).
  - per-sample VLAD accumulators live in ONE PSUM bank as [d, 4, k]
    regions plus an asum bank, zeroed ONCE by DVE memsets; every
    matmul accumulates with start=False (skip_group_check).
  - softmax is batched per PAIR of groups (8 tiles / 1024 pixels):
    ONE ACT Exp [128, 8, 32] straight out of the mm1 PSUM pair-tile,
    ONE DVE reduce -> Z, ONE DVE reciprocal, ONE Pool tensor_tensor
    multiply with 1/Z broadcast via a stride-0 AP -> a bf16.
  - mm2: 16 matmuls/group acc[d,k] += x_chunk.T @ a plus asum matmuls,
    sample-split partition ranges at boundary tile 24.
  - epilogue per sample: diag(asum) trick folds C*asum into PSUM via 4
    PE matmuls carrying the stop; ONE Square + ONE DVE reduce for ssq;
    1/sqrt(D*ssq) = exp(-0.5*ln(D*ssq)) on ACT (shared Exp+Ln table,
    pinned once); ONE broadcast tensor_tensor scales straight out of
    PSUM; one 512B-per-partition output DMA per sample.
NOTE: dram params MUST receive numpy arrays of the declared dtype
(ml_dtypes); f32 arrays are silently reinterpreted and produce NaN.
"""
import sys

if '/opt/trn_rl_repo' not in sys.path:
    sys.path.insert(0, '/opt/trn_rl_repo')

from contextlib import ExitStack

import numpy as np

N_PIX = 3136
N_SAMP = 2
N_ROWS = N_PIX * N_SAMP
P = 128
NT = N_ROWS // P      # 49
D = 512
K = 32
DC = D // P           # 4
G = 4                 # tiles per group
NG = NT // G          # 12 full groups
RUNT = NT - NG * G    # 1
BOUND_T = N_PIX // P  # 24 (tile 24 is split 64/64 between samples)
BOUND_R = N_PIX - BOUND_T * P  # 64
N_CORES = 8

# packed-constants column layout (bf16 elements)
CPK_WC = 0            # [128, 128]: wc[p, c*32+k] = Wc[c*128+p, k]
CPK_CT = 128          # [32, 512]:  ct[k, d] = C[d, k]
CPK_ID32 = 640        # [32, 32]:   eye
CPK_ONES = 672        # [128, 2]:   ones
CPK_ID128 = 674       # [128, 128]: eye (transpose moving operand)
CPK_W = 802

_cache = {}


def _group_tiles(g):
    return (G, g * G) if g < NG else (RUNT, NG * G)


def _build():
    import concourse.bacc as bacc
    import concourse.mybir as mybir
    import concourse.tile as tile
    from concourse.bass import ts

    F32 = mybir.dt.float32
    BF16 = mybir.dt.bfloat16
    FP8 = mybir.dt.float8e3

    nc = bacc.Bacc("TRN2", target_bir_lowering=False, debug=False)

    x = nc.declare_dram_parameter("x", [N_ROWS, D], FP8, isOutput=False)
    xt = nc.declare_dram_parameter("xt", [NG * D, (G + RUNT) * P],
                                   FP8, isOutput=False)
    cpk = nc.declare_dram_parameter("cpk", [P, CPK_W], BF16, isOutput=False)
    out = nc.declare_dram_parameter("out", [N_SAMP, P, DC, K], F32,
                                    isOutput=True)
    x, xt, cpk, out = x.ap(), xt.ap(), cpk.ap(), out.ap()

    # greedy DMA queue balancer, seeded with each queue's compute load
    # (scalar carries the exps + act-table loads + epilogue activations;
    # gpsimd only the consts; sync only an output half)
    qload = dict(_cache.get("seeds", {"sync": 500.0, "scalar": 3000.0,
                                      "gpsimd": 1000.0}))

    def dma(out_, in_, cost, queue=None):
        if queue is None:
            queue = min(qload, key=qload.get)
        qload[queue] += max(cost, 500.0)
        getattr(nc, queue).dma_start(out=out_, in_=in_)

    with tile.TileContext(nc) as tc, ExitStack() as ctx:
        consts = ctx.enter_context(tc.tile_pool(name="consts", bufs=1))
        xbig = ctx.enter_context(tc.tile_pool(name="xbig", bufs=1))
        xtd = ctx.enter_context(tc.tile_pool(name="xtd", bufs=1))
        small = ctx.enter_context(tc.tile_pool(name="small", bufs=4))
        epil = ctx.enter_context(tc.tile_pool(name="epil", bufs=2))
        ps_s = ctx.enter_context(tc.tile_pool(name="ps_s", bufs=2,
                                              space="PSUM"))
        ps_xt = ctx.enter_context(tc.tile_pool(name="ps_xt", bufs=2,
                                               space="PSUM"))
        ps_acc = ctx.enter_context(tc.tile_pool(name="ps_acc", bufs=1,
                                                space="PSUM"))

        # one packed bf16 constants DMA; everything else is a view
        cpk_sb = consts.tile([P, CPK_W], BF16)
        dma(cpk_sb, cpk, CPK_W * 2 * 0.3855, queue="gpsimd")
        wc_sb = cpk_sb[:, CPK_WC:CPK_WC + DC * K].rearrange(
            "p (c k) -> p c k", c=DC)
        ct_sb = cpk_sb[0:K, CPK_CT:CPK_CT + D]
        id32_sb = cpk_sb[0:K, CPK_ID32:CPK_ID32 + K]
        ones_sb = cpk_sb[:, CPK_ONES:CPK_ONES + 2]
        id128_sb = cpk_sb[:, CPK_ID128:CPK_ID128 + P]

        NSG = NG  # 12 storage groups; the last one carries 5 tiles
        xgS = []
        xtS = []
        for sg in range(NSG):
            n = G if sg < NSG - 1 else G + RUNT
            xgS.append(xbig.tile([P, n, D], FP8, name=f"xg{sg}"))
            xtS.append(xtd.tile([P, DC, n * P], FP8, name=f"xt{sg}"))

        def xg_view(g):
            if g < NSG - 1:
                return xgS[g]
            return xgS[NSG - 1][:, 0:G, :] if g == NSG - 1 \
                else xgS[NSG - 1][:, G:G + RUNT, :]

        TPOSE = tuple(_cache.get("tpose", ()))
        xtT = {g: xtd.tile([P, DC, G * P], BF16, name=f"xtT{g}")
               for g in TPOSE}

        def xt_view(g):
            if g in TPOSE:
                return xtT[g]
            if g < NSG - 1:
                return xtS[g]
            return xtS[NSG - 1][:, :, 0:G * P] if g == NSG - 1 \
                else xtS[NSG - 1][:, :, G * P:(G + RUNT) * P]

        def stage_T(g):
            # on-chip xT for early groups: normal-mode PE matmuls
            # against a bf16 identity (fp8 is_transpose is
            # verifier-rejected) -> f32 PSUM, drained per tile by a
            # converting DVE copy to bf16; trades idle early PE/DVE
            # time for ~0.8us of DMA queue time per group
            for i in range(G):
                t_ps = ps_xt.tile([P, DC, P], F32, name="xT_ps")
                for c in range(DC):
                    nc.tensor.matmul(t_ps[:, c, :],
                                     xg_view(g)[:, i, ts(c, P)],
                                     id128_sb, start=True, stop=True)
                nc.vector.tensor_copy(xtT[g][:, :, ts(i, P)], t_ps)

        def dma_xt(sg, queue=None):
            if sg in TPOSE:
                return
            n = G if sg < NSG - 1 else G + RUNT
            dma(xtS[sg],
                xt[sg * D:(sg + 1) * D, 0:n * P].rearrange(
                    "(c p) q -> p c q", p=P),
                DC * n * P * 0.3855, queue=queue)

        def dma_xg(sg, queue=None):
            n = G if sg < NSG - 1 else G + RUNT
            r0 = sg * G * P
            dma(xgS[sg],
                x[r0:r0 + n * P, :].rearrange("(j p) d -> p j d", p=P),
                n * D * 0.3855, queue=queue)

        # per-sample [d, 4, k] accumulators as SEPARATE PSUM tiles (so
        # sample 0's epilogue readers don't false-serialize against
        # sample 1's accumulation writers), zeroed once by DVE memsets;
        # all matmuls accumulate with start=False (per-element HW
        # semantics match plain writes)
        ps_acc_t = [ps_acc.tile([P, DC * K], F32, name=f"acc{s}",
                                tag=f"acc{s}") for s in range(N_SAMP)]
        asums_t = [ps_acc.tile([K, 2], F32, name=f"asum{s}",
                               tag=f"asumps{s}") for s in range(N_SAMP)]
        for s in range(N_SAMP):
            nc.vector.memset(ps_acc_t[s], 0.0)
            nc.vector.memset(asums_t[s], 0.0)

        def acc_jk(s, j):
            return ps_acc_t[s][:, j * K:(j + 1) * K]

        def acc_s(s):
            return ps_acc_t[s].rearrange("p (c k) -> p c k", c=DC)

        asum_ps = asums_t

        diags = {}

        def epilogue_pre(s):
            asum_sb = epil.tile([K, 1], F32, name=f"asum{s}", tag="asum")
            nc.vector.tensor_copy(asum_sb, asum_ps[s][:, 0:1])
            # diag(asum) = id32 * asum (per-partition scalar), bf16 for PE
            diag = epil.tile([K, K], BF16, name=f"diag{s}", tag="diag")
            nc.vector.tensor_scalar_mul(diag, id32_sb, asum_sb)
            diags[s] = diag

        def epilogue_post(s):
            # fold the C*asum term into the PSUM accumulation:
            # acc[s] chunk j += ct_chunk.T @ diag(asum); these carry the
            # stop of the accumulation groups
            diag = diags.pop(s)
            for j in range(DC):
                nc.tensor.matmul(acc_jk(s, j), ct_sb[:, ts(j, P)], diag,
                                 start=False, stop=True,
                                 skip_group_check=True)
            # rmult = exp(-0.5*ln(D*ssq)) = 1/sqrt(D*ssq); ln+exp share
            # one act func set (no Sqrt set switch). Sample 0 runs one
            # monolithic chain (fewer ACT ops queued ahead of the
            # remaining loop exps on the in-order ACT queue); sample 1
            # -- the tail -- runs two pipelined half-chains so the
            # first output half DMAs while the second still scales.
            v_sb = epil.tile([P, DC, K], F32, name=f"v{s}", tag="v")
            vsq = epil.tile([P, DC, K], F32, name=f"vsq{s}", tag="vsq")
            ssq = epil.tile([P, DC], F32, name=f"ssq{s}", tag="ssq")
            lssq = epil.tile([P, DC], F32, name=f"ls{s}", tag="ls")
            rmult = epil.tile([P, DC, 1], F32, name=f"rm{s}", tag="rm")
            halves = [slice(0, DC)]
            if s == 1 and _cache.get("split_s1", False):
                halves = [slice(0, 2), slice(2, DC)]
            for h, hj in enumerate(halves):
                w = hj.stop - hj.start
                nc.scalar.activation(vsq[:, hj, :], acc_s(s)[:, hj, :],
                                     mybir.ActivationFunctionType.Square)
                nc.vector.reduce_sum(ssq[:, hj], vsq[:, hj, :],
                                     axis=mybir.AxisListType.X)
                nc.scalar.activation(lssq[:, hj], ssq[:, hj],
                                     mybir.ActivationFunctionType.Ln,
                                     scale=float(D))
                nc.scalar.activation(rmult[:, hj, 0], lssq[:, hj],
                                     mybir.ActivationFunctionType.Exp,
                                     scale=-0.5)
                nc.vector.tensor_tensor(v_sb[:, hj, :], acc_s(s)[:, hj, :],
                                        rmult[:, hj, :].broadcast_to(
                                            [P, w, K]),
                                        op=mybir.AluOpType.mult)
                dma(out[s][:, hj, :], v_sb[:, hj, :], 500,
                    queue=("gpsimd", "sync")[h] if s == 1 else "gpsimd")

        SB = 4  # groups per softmax batch
        s_pairs = {}
        a_pairs = {}

        def stage_M1(g):
            n, t0 = _group_tiles(g)
            p, half = g // SB, g % SB
            if p not in s_pairs:
                w = SB * G if g < NG else RUNT
                s_pairs[p] = ps_s.tile([P, w, K], F32, name="spair",
                                       tag="sp")
            s_ps = s_pairs[p]
            xT_sb = xt_view(g)
            for i in range(n):
                for c in range(DC):
                    nc.tensor.matmul(s_ps[:, half * G + i, :],
                                     xT_sb[:, c, ts(i, P)],
                                     wc_sb[:, c, :],
                                     start=(c == 0), stop=(c == DC - 1))

        def stage_SM(p):
            # batched softmax over four groups (16 tiles): one Exp, one
            # reduce, one reciprocal, one broadcast multiply
            s_ps = s_pairs.pop(p)
            w = s_ps.shape[1]
            exp_sb = small.tile([P, w, K], F32, name="exp_sb")
            zs = small.tile([P, w], F32, name="zs")
            rz = small.tile([P, w, 1], F32, name="rz")
            a_sb = small.tile([P, w, K], BF16, name="a_sb")
            nc.scalar.activation(exp_sb, s_ps,
                                 mybir.ActivationFunctionType.Exp)
            # Z, 1/Z and the divide all stay on DVE: back-to-back on one
            # queue beats hopping to Pool (two fewer semaphore waits on
            # the a-chain), and it frees Pool for pure DMA work
            nc.vector.reduce_sum(zs, exp_sb, axis=mybir.AxisListType.X)
            nc.vector.reciprocal(rz[:, :, 0], zs)
            norm_eng = nc.vector if _cache.get("norm_dve", False) \
                else nc.gpsimd
            norm_eng.tensor_tensor(a_sb, exp_sb,
                                   rz.broadcast_to([P, w, K]),
                                   op=mybir.AluOpType.mult)
            a_pairs[p] = a_sb

        def stage_M2(g):
            n, t0 = _group_tiles(g)
            p, half = g // SB, g % SB
            a_sb = a_pairs[p]
            for i in range(n):
                t = t0 + i
                ai = a_sb[:, half * G + i, :]
                if t < BOUND_T:
                    parts = [(0, 0, P)]
                elif t == BOUND_T:
                    parts = [(0, 0, BOUND_R), (1, BOUND_R, P)]
                else:
                    parts = [(1, 0, P)]
                for s, r0, r1 in parts:
                    # sample 1 closes on tile 47 (group 11), which is
                    # EMITTED last: the runt group 12 is processed early
                    # so the closing chain isn't gated on the final DMA
                    last_tile = (t == BOUND_T and s == 0) or \
                                (t == NT - 1 - RUNT - G and s == 1)
                    def asum_mm():
                        nc.tensor.matmul(asum_ps[s][:, :], ai[r0:r1, :],
                                         ones_sb[r0:r1, :],
                                         start=False, stop=last_tile,
                                         skip_group_check=True)
                        if last_tile:
                            epilogue_pre(s)

                    def acc_mms():
                        for c in range(DC):
                            nc.tensor.matmul(acc_jk(s, c),
                                             xg_view(g)[r0:r1, i, ts(c, P)],
                                             ai[r0:r1, :],
                                             start=False, stop=False,
                                             skip_group_check=True)
                    # asum first: on the last tile the DVE diag chain
                    # (epilogue_pre) overlaps the remaining acc matmuls
                    if _cache.get("asum_first", True):
                        asum_mm(); acc_mms()
                    else:
                        acc_mms(); asum_mm()

        # all xT streams issue ahead of the x-naturals: every softmax
        # chain then completes as early as its transfer lands, and the
        # kernel's closing chain is gated only on the LAST x-natural --
        # ordered to be group 10's, whose M2 is emitted last (xt0 on SP
        # because the scheduler runs the act-table pin first on ACT)
        dma_xt(0, "sync")
        dma_xt(1, "scalar")
        dma_xt(2, "gpsimd")
        dma_xg(0, "sync")
        for g in sorted(TPOSE):
            if g > 0:
                dma_xg(g, None)
            stage_T(g)

        # pin the one act table covering Exp+Ln+Square up front so the
        # fixpoint table-load pass doesn't thrash between the exp-only
        # and ln-only sets at the epilogues (without this it inserts 5
        # loads, 6.4us of ACT time)
        try:
            from concourse.hw_specs import get_activation_tables
            sets = get_activation_tables(nc.m.arch)
            _EXP = mybir.ActivationFunctionType.Exp
            _LN = mybir.ActivationFunctionType.Ln
            set_id = next(i for i, name in enumerate(sets)
                          if _EXP in sets[name] and _LN in sets[name])
            nc.scalar.add_instruction(
                mybir.InstLoadActFuncSet(
                    name=nc.get_next_instruction_name(), ins=[], outs=[],
                    act_func_set_id=set_id))
        except Exception:
            pass

        # M2 emission order [0..9, 11, 12, 10]: group 10's accumulation
        # closes sample 1 and is the only chain gated on the final DMA
        m2_order = list(range(NG - 2)) + [NG - 1, NG, NG - 2]
        if _cache.get("xg_late", True):
            xg_order = list(range(1, NSG - 2)) + [NSG - 1, NSG - 2]
        else:
            xg_order = list(range(1, NSG))
        xg_order = [g for g in xg_order if g not in TPOSE or g == 0]
        lead = _cache.get("xt_lead", 5)
        for t in range(NG + 6):
            if 0 <= t - 1 <= NG:
                g = t - 1
                stage_M1(g)
                if g % 4 == 3 or g == NG:
                    stage_SM(g // 4)
            if 0 <= t - 5 <= NG:
                stage_M2(m2_order[t - 5])
            if t + 3 < NSG:
                dma_xt(t + 3)
            if 0 <= t - lead < len(xg_order):
                dma_xg(xg_order[t - lead])
            if t - 5 == BOUND_T // G + 2:
                epilogue_post(0)
            if t - 5 == NG:
                epilogue_post(1)

    # the single pinned set-6 load covers every activation we emit
    # (Exp, Ln, Square); the auto pass would add a redundant 1.3us load
    nc.insert_act_table_loads = lambda: None
    nc.finalize()
    return nc


def _get_nc():
    if "nc" not in _cache:
        _cache["nc"] = _build()
    return _cache["nc"]


def _make_maps(x, Wc, C):
    import ml_dtypes
    bf16 = ml_dtypes.bfloat16
    fp8 = ml_dtypes.float8_e3m4
    x = np.asarray(x, dtype=np.float32)
    Wc = np.asarray(Wc, dtype=np.float32)
    C = np.asarray(C, dtype=np.float32)

    cpk = np.zeros((P, CPK_W), dtype=np.float32)
    cpk[:, CPK_WC:CPK_WC + DC * K] = Wc.reshape(DC, P, K).transpose(
        1, 0, 2).reshape(P, DC * K)
    cpk[0:K, CPK_CT:CPK_CT + D] = C.T
    cpk[0:K, CPK_ID32:CPK_ID32 + K] = np.eye(K, dtype=np.float32)
    cpk[:, CPK_ONES:CPK_ONES + 2] = 1.0
    cpk[:, CPK_ID128:CPK_ID128 + P] = np.eye(P, dtype=np.float32)
    cpk = cpk.astype(bf16)

    B = x.shape[0]
    per = B // N_CORES
    maps = []
    for i in range(N_CORES):
        xs = np.ascontiguousarray(
            x[i * per:(i + 1) * per].reshape(N_ROWS, D).astype(fp8))
        W = (G + RUNT) * P
        blocks = []
        for sg in range(NG):
            hi = min((sg + 1) * G * P + (RUNT * P if sg == NG - 1 else 0),
                     N_ROWS)
            blk = xs[sg * G * P:hi].T
            if blk.shape[1] < W:
                pad = np.zeros((D, W - blk.shape[1]), dtype=blk.dtype)
                blk = np.concatenate([blk, pad], axis=1)
            blocks.append(np.ascontiguousarray(blk))
        maps.append({"x": xs, "xt": np.concatenate(blocks, axis=0),
                     "cpk": cpk})
    return maps


def kernel(x, Wc, C):
    from concourse.bass_utils import run_bass_kernel_spmd

    nc = _get_nc()
    maps = _make_maps(x, Wc, C)
    res = run_bass_kernel_spmd(nc, maps, list(range(N_CORES)))
    outs = [r["out"].transpose(0, 2, 1, 3).reshape(N_SAMP, D * K)
            for r in res.results]
    return np.concatenate(outs, axis=0)
